# revision 1
# baseline (speedup 1.0000x reference)
"""Trainium2 Bass kernel for nn_Apply_Mask (topk_masking).

Reference semantics, per (batch, channel) slice of shape 32x32:
  - find argmax location (mh, mw)
  - build clipped 5x5 box around it; S = 1 - box
  - lam = 1024 / sum(S)
  - out = T != 0 ? x * S * lam : x

Sharding: embarrassingly data-parallel over the 32768 (b*c) slices;
core i takes slices [4096*i, 4096*(i+1)).

Per-core layout: partition p holds 32 slices [32p, 32p+32) along the free
dim; tile t = slice 32p+t at free offset t*1024. All compute is exact f32:
  argmax via DVE max/max_index, box mask via iota comparisons,
  scale = a - sel*lam*box applied with one scalar_tensor_tensor per tile.
"""
import sys

for _p in ("/opt/trn_rl_repo",):
    if _p not in sys.path:
        sys.path.insert(0, _p)

import numpy as np

import concourse.bass as bass
import concourse.tile as tile
from concourse import bacc, mybir
from concourse.bass_utils import run_bass_kernel_spmd

P = 128          # partitions
NT = 32          # tiles (slices) per partition
H = W = 32
HW = H * W
NCHUNK = 8       # DMA chunks (NT/4 tiles each)
CH_T = NT // NCHUNK   # tiles per chunk
N_CORES = 8
SLICES_PER_CORE = P * NT  # 4096

f32 = mybir.dt.float32
u16 = mybir.dt.uint16
Alu = mybir.AluOpType

_cached = {}


def _build(half: int):
    nc = bacc.Bacc("TRN2", target_bir_lowering=False, debug=False,
                   num_devices=N_CORES)
    x_in = nc.dram_tensor("x", [P, NT * HW], f32, kind="ExternalInput").ap()
    sel_in = nc.dram_tensor("sel", [P, NT], f32, kind="ExternalInput").ap()
    io_in = nc.dram_tensor("io32", [P, 32], f32, kind="ExternalInput").ap()
    out_d = nc.dram_tensor("out", [P, NT * HW], f32, kind="ExternalOutput").ap()

    with tile.TileContext(nc) as tc:
        from contextlib import ExitStack
        with ExitStack() as ctx:
            xpool = ctx.enter_context(tc.tile_pool(name="xp", bufs=1))
            mid = ctx.enter_context(tc.tile_pool(name="mid", bufs=1))
            small = ctx.enter_context(tc.tile_pool(name="small", bufs=1))
            qpool = ctx.enter_context(tc.tile_pool(name="qp", bufs=3))
            opool = ctx.enter_context(tc.tile_pool(name="op", bufs=2))

            # ---- input DMA in NCHUNK chunks ----
            xc = []
            for c in range(NCHUNK):
                t_ = xpool.tile([P, CH_T * HW], f32, name=f"x{c}", tag=f"x{c}")
                nc.sync.dma_start(t_[:], x_in[:, c * CH_T * HW:(c + 1) * CH_T * HW])
                xc.append(t_)

            def x_tile(t):
                c, j = divmod(t, CH_T)
                return xc[c][:, j * HW:(j + 1) * HW]

            selp = small.tile([P, NT], f32)
            nc.sync.dma_start(selp[:], sel_in)
            io32 = small.tile([P, 32], f32)
            nc.sync.dma_start(io32[:], io_in)

            # ---- per-tile argmax (value top8 + index) ----
            max8 = mid.tile([P, NT, 8], f32)
            idx8 = mid.tile([P, NT, 8], u16)
            for t in range(NT):
                nc.vector.max(max8[:, t], x_tile(t))
                nc.vector.max_index(idx8[:, t], max8[:, t], x_tile(t))

            # ---- batched per-slice scalar math ([P, NT]) ----
            idx_u = small.tile([P, NT], u16)
            nc.vector.tensor_copy(idx_u[:], idx8[:, :, 0])
            mh_u = small.tile([P, NT], u16)
            mw_u = small.tile([P, NT], u16)
            nc.vector.tensor_scalar(mh_u[:], idx_u[:], 5, None, Alu.logical_shift_right)
            nc.vector.tensor_scalar(mw_u[:], idx_u[:], 31, None, Alu.bitwise_and)
            mh = small.tile([P, NT], f32)
            mw = small.tile([P, NT], f32)
            nc.vector.tensor_copy(mh[:], mh_u[:])
            nc.vector.tensor_copy(mw[:], mw_u[:])

            h1 = small.tile([P, NT], f32)
            h2 = small.tile([P, NT], f32)
            w1 = small.tile([P, NT], f32)
            w2 = small.tile([P, NT], f32)
            nc.vector.tensor_scalar(h1[:], mh[:], float(half), 0.0, Alu.subtract, Alu.max)
            nc.vector.tensor_scalar(h2[:], mh[:], float(half), float(H - 1), Alu.add, Alu.min)
            nc.vector.tensor_scalar(w1[:], mw[:], float(half), 0.0, Alu.subtract, Alu.max)
            nc.vector.tensor_scalar(w2[:], mw[:], float(half), float(W - 1), Alu.add, Alu.min)

            rl = small.tile([P, NT], f32)
            cl1 = small.tile([P, NT], f32)
            area = small.tile([P, NT], f32)
            nc.vector.tensor_tensor(rl[:], h2[:], h1[:], Alu.subtract)
            nc.vector.tensor_tensor(cl1[:], w2[:], w1[:], Alu.subtract)
            nc.vector.tensor_scalar(cl1[:], cl1[:], 1.0, None, Alu.add)
            nc.vector.scalar_tensor_tensor(area[:], rl[:], 1.0, cl1[:], Alu.add, Alu.mult)

            denom = small.tile([P, NT], f32)
            nc.vector.tensor_scalar(denom[:], area[:], -1.0, float(HW), Alu.mult, Alu.add)
            recip = small.tile([P, NT], f32)
            nc.vector.reciprocal(recip[:], denom[:])
            lam1 = small.tile([P, NT], f32)      # lam - 1
            nc.vector.tensor_scalar(lam1[:], recip[:], float(HW), -1.0, Alu.mult, Alu.add)
            a_t = small.tile([P, NT], f32)       # a = 1 + sel*(lam-1)
            nc.vector.scalar_tensor_tensor(a_t[:], lam1[:], 0.0, selp[:], Alu.add, Alu.mult)
            nc.vector.tensor_scalar(a_t[:], a_t[:], 1.0, None, Alu.add)
            nb_t = small.tile([P, NT], f32)      # -sel*lam = -(a - 1 + sel)
            nc.vector.scalar_tensor_tensor(nb_t[:], a_t[:], 1.0, selp[:], Alu.subtract, Alu.add)
            nc.vector.tensor_scalar(nb_t[:], nb_t[:], -1.0, None, Alu.mult)

            # ---- mask mids: col_in [P,NT,W], row_nb [P,NT,H] ----
            io_b = io32[:, None, :].broadcast_to([P, NT, 32])
            col_in = mid.tile([P, NT, W], f32)
            col_gt = mid.tile([P, NT, W], f32)
            nc.vector.tensor_tensor(col_in[:], io_b, w1[:, :, None].broadcast_to([P, NT, W]), Alu.is_ge)
            nc.vector.tensor_tensor(col_gt[:], io_b, w2[:, :, None].broadcast_to([P, NT, W]), Alu.is_gt)
            nc.vector.tensor_tensor(col_in[:], col_in[:], col_gt[:], Alu.subtract)

            row_nb = mid.tile([P, NT, H], f32)
            row_gt = mid.tile([P, NT, H], f32)
            nc.vector.tensor_tensor(row_nb[:], io_b, h1[:, :, None].broadcast_to([P, NT, H]), Alu.is_ge)
            nc.vector.tensor_tensor(row_gt[:], io_b, h2[:, :, None].broadcast_to([P, NT, H]), Alu.is_gt)
            nc.vector.tensor_tensor(row_nb[:], row_nb[:], row_gt[:], Alu.subtract)
            nc.vector.tensor_tensor(row_nb[:], row_nb[:], nb_t[:, :, None].broadcast_to([P, NT, H]), Alu.mult)

            # ---- apply + output DMA, chunked ----
            for c in range(NCHUNK):
                o_c = opool.tile([P, CH_T * HW], f32, name=f"o{c}", tag="oc")
                for j in range(CH_T):
                    t = c * CH_T + j
                    q = qpool.tile([P, H, W], f32, name=f"q{t}", tag="q")
                    nc.vector.tensor_tensor(
                        q[:],
                        row_nb[:, t, :, None].broadcast_to([P, H, W]),
                        col_in[:, t, None, :].broadcast_to([P, H, W]),
                        Alu.mult,
                    )
                    # out = (q + a) * x
                    nc.vector.scalar_tensor_tensor(
                        o_c[:, j * HW:(j + 1) * HW].rearrange("p (h w) -> p h w", h=H, w=W),
                        q[:], a_t[:, t, None],
                        x_tile(t).rearrange("p (h w) -> p h w", h=H, w=W),
                        Alu.add, Alu.mult,
                    )
                nc.sync.dma_start(out_d[:, c * CH_T * HW:(c + 1) * CH_T * HW], o_c[:])

    nc.compile()
    return nc


def _get_nc(half: int):
    if half not in _cached:
        _cached[half] = _build(half)
    return _cached[half]


def _shard_inputs(x, T):
    xf = np.ascontiguousarray(x, dtype=np.float32).reshape(-1, HW)   # [32768, 1024]
    sel = (np.asarray(T).reshape(-1) != 0).astype(np.float32)        # [32768]
    io32 = np.tile(np.arange(32, dtype=np.float32), (P, 1))
    in_maps = []
    for i in range(N_CORES):
        lo = i * SLICES_PER_CORE
        hi = lo + SLICES_PER_CORE
        in_maps.append({
            "x": np.ascontiguousarray(xf[lo:hi].reshape(P, NT * HW)),
            "sel": np.ascontiguousarray(sel[lo:hi].reshape(P, NT)),
            "io32": io32,
        })
    return in_maps


def run(inputs, trace=False, **kw):
    x = inputs["x"]
    T = inputs["T"]
    drop_block = int(np.asarray(inputs["drop_block"]))
    half = drop_block // 2
    b, c, h, w = x.shape
    assert (h, w) == (H, W) and b * c == N_CORES * SLICES_PER_CORE, \
        f"kernel hardcoded for (128,256,32,32); got {x.shape}"

    nc = _get_nc(half)
    in_maps = _shard_inputs(x, T)
    res = run_bass_kernel_spmd(nc, in_maps, core_ids=list(range(N_CORES)),
                               trace=trace, **kw)
    parts = [res.results[i]["out"].reshape(SLICES_PER_CORE, HW)
             for i in range(N_CORES)]
    out = np.concatenate(parts, axis=0).reshape(b, c, h, w).astype(np.float32)
    return out, res


def kernel(**inputs) -> np.ndarray:
    out, _ = run(inputs, trace=False)
    return out


# revision 8
# speedup vs baseline: 1.1278x; 1.1278x over previous
"""Trainium2 Bass kernel for nn_Apply_Mask (topk_masking).

Reference semantics, per (batch, channel) slice of shape 32x32:
  - find argmax location (mh, mw)
  - build clipped 5x5 box around it; S = 1 - box
  - lam = 1024 / sum(S)
  - out = T != 0 ? x * S * lam : x

Sharding: embarrassingly data-parallel over the 32768 (b*c) slices;
core i takes slices [4096*i, 4096*(i+1)).

Per-core layout: partition p holds 32 slices [32p, 32p+32) along the free
dim; tile t = slice 32p+t at free offset t*1024.

Engine split (fast path):
  DVE    : exact f32 argmax (max8/find_index8), per-slice scalar math,
           final select out = (q==0)*y as all-bf16 scalar_tensor_tensor
  ScalarE: y = a*x (activation copy with per-partition scale, bf16 out)
  GpSimd : q = row_nb (x) col_in outer product (bf16 tensor_tensor)
  DMA    : f32 in (16 MiB), bf16 out (8 MiB)
"""
import sys

for _p in ("/opt/trn_rl_repo",):
    if _p not in sys.path:
        sys.path.insert(0, _p)

import numpy as np

import concourse.bass as bass
import concourse.tile as tile
from concourse import bacc, mybir
from concourse.bass_utils import run_bass_kernel_spmd

P = 128          # partitions
NT = 32          # tiles (slices) per partition
H = W = 32
HW = H * W
NCHUNK = 8       # DMA chunks (NT/NCHUNK tiles each)
CH_T = NT // NCHUNK
N_CORES = 8
SLICES_PER_CORE = P * NT  # 4096

EXACT = True      # True: all-f32 exact path; False: bf16 apply (rel err ~2e-3)
OUT_BF16 = True   # write output as bf16 (halves output DMA; rel err ~2e-3)
KQ = 4            # tiles per outer-product instruction
POOL_QBATCHES = 5 # how many of the NT/KQ outer batches run on GpSimd

f32 = mybir.dt.float32
bf16 = mybir.dt.bfloat16
u16 = mybir.dt.uint16
Alu = mybir.AluOpType
Act = mybir.ActivationFunctionType

_cached = {}


def _build(half: int):
    mdt = f32 if EXACT else bf16   # mask / q dtype
    odt = bf16 if (OUT_BF16 or not EXACT) else f32   # output dtype

    nc = bacc.Bacc("TRN2", target_bir_lowering=False, debug=False,
                   num_devices=N_CORES)
    x_in = nc.dram_tensor("x", [P, NT * HW], f32, kind="ExternalInput").ap()
    sel_in = nc.dram_tensor("sel", [P, NT], f32, kind="ExternalInput").ap()
    io_in = nc.dram_tensor("io32", [P, 32], f32, kind="ExternalInput").ap()
    out_d = nc.dram_tensor("out", [P, NT * HW], odt, kind="ExternalOutput").ap()

    with tile.TileContext(nc) as tc:
        from contextlib import ExitStack
        with ExitStack() as ctx:
            xpool = ctx.enter_context(tc.tile_pool(name="xp", bufs=1))
            mid = ctx.enter_context(tc.tile_pool(name="mid", bufs=1))
            small = ctx.enter_context(tc.tile_pool(name="small", bufs=1))
            qpool = ctx.enter_context(tc.tile_pool(name="qp", bufs=2))
            ypool = ctx.enter_context(tc.tile_pool(name="yp", bufs=4))
            opool = ctx.enter_context(tc.tile_pool(name="op", bufs=2))

            # ---- input DMA in NCHUNK chunks ----
            xc = []
            for c in range(NCHUNK):
                t_ = xpool.tile([P, CH_T * HW], f32, name=f"x{c}", tag=f"x{c}")
                nc.sync.dma_start(t_[:], x_in[:, c * CH_T * HW:(c + 1) * CH_T * HW])
                xc.append(t_)

            def x_tile(t):
                c, j = divmod(t, CH_T)
                return xc[c][:, j * HW:(j + 1) * HW]

            selp = small.tile([P, NT], f32)
            nc.sync.dma_start(selp[:], sel_in)
            io32 = small.tile([P, 32], f32)
            nc.sync.dma_start(io32[:], io_in)

            # ---- per-tile argmax (value top8 + index) ----
            max8 = mid.tile([P, NT, 8], f32)
            idx8 = mid.tile([P, NT, 8], u16)
            for t in range(NT):
                nc.vector.max(max8[:, t], x_tile(t))
                nc.vector.max_index(idx8[:, t], max8[:, t], x_tile(t))

            # ---- batched per-slice scalar math ([P, NT]) ----
            idx_u = small.tile([P, NT], u16)
            nc.vector.tensor_copy(idx_u[:], idx8[:, :, 0])
            mh_u = small.tile([P, NT], u16)
            mw_u = small.tile([P, NT], u16)
            nc.vector.tensor_scalar(mh_u[:], idx_u[:], 5, None, Alu.logical_shift_right)
            nc.vector.tensor_scalar(mw_u[:], idx_u[:], 31, None, Alu.bitwise_and)
            mh = small.tile([P, NT], f32)
            mw = small.tile([P, NT], f32)
            nc.vector.tensor_copy(mh[:], mh_u[:])
            nc.vector.tensor_copy(mw[:], mw_u[:])

            h1 = small.tile([P, NT], f32)
            h2 = small.tile([P, NT], f32)
            w1 = small.tile([P, NT], f32)
            w2 = small.tile([P, NT], f32)
            nc.vector.tensor_scalar(h1[:], mh[:], float(half), 0.0, Alu.subtract, Alu.max)
            nc.vector.tensor_scalar(h2[:], mh[:], float(half), float(H - 1), Alu.add, Alu.min)
            nc.vector.tensor_scalar(w1[:], mw[:], float(half), 0.0, Alu.subtract, Alu.max)
            nc.vector.tensor_scalar(w2[:], mw[:], float(half), float(W - 1), Alu.add, Alu.min)

            rl = small.tile([P, NT], f32)
            cl1 = small.tile([P, NT], f32)
            area = small.tile([P, NT], f32)
            nc.vector.tensor_tensor(rl[:], h2[:], h1[:], Alu.subtract)
            nc.vector.tensor_tensor(cl1[:], w2[:], w1[:], Alu.subtract)
            nc.vector.tensor_scalar(cl1[:], cl1[:], 1.0, None, Alu.add)
            nc.vector.scalar_tensor_tensor(area[:], rl[:], 1.0, cl1[:], Alu.add, Alu.mult)

            denom = small.tile([P, NT], f32)
            nc.vector.tensor_scalar(denom[:], area[:], -1.0, float(HW), Alu.mult, Alu.add)
            recip = small.tile([P, NT], f32)
            nc.vector.reciprocal(recip[:], denom[:])
            lam1 = small.tile([P, NT], f32)      # lam - 1
            nc.vector.tensor_scalar(lam1[:], recip[:], float(HW), -1.0, Alu.mult, Alu.add)
            a_t = small.tile([P, NT], f32)       # a = 1 + sel*(lam-1)
            nc.vector.scalar_tensor_tensor(a_t[:], lam1[:], 0.0, selp[:], Alu.add, Alu.mult)
            nc.vector.tensor_scalar(a_t[:], a_t[:], 1.0, None, Alu.add)
            nb_t = small.tile([P, NT], f32)      # -sel*lam = -(a - 1 + sel)
            nc.vector.scalar_tensor_tensor(nb_t[:], a_t[:], 1.0, selp[:], Alu.subtract, Alu.add)
            nc.vector.tensor_scalar(nb_t[:], nb_t[:], -1.0, None, Alu.mult)

            # ---- mask mids: col_in [P,NT,W], row_nb [P,NT,H] ----
            io_b = io32[:, None, :].broadcast_to([P, NT, 32])
            col_in = mid.tile([P, NT, W], mdt)
            col_gt = mid.tile([P, NT, W], mdt)
            nc.vector.tensor_tensor(col_in[:], io_b, w1[:, :, None].broadcast_to([P, NT, W]), Alu.is_ge)
            nc.vector.tensor_tensor(col_gt[:], io_b, w2[:, :, None].broadcast_to([P, NT, W]), Alu.is_gt)
            nc.vector.tensor_tensor(col_in[:], col_in[:], col_gt[:], Alu.subtract)

            row_nb = mid.tile([P, NT, H], mdt)
            row_gt = mid.tile([P, NT, H], mdt)
            nc.vector.tensor_tensor(row_nb[:], io_b, h1[:, :, None].broadcast_to([P, NT, H]), Alu.is_ge)
            nc.vector.tensor_tensor(row_gt[:], io_b, h2[:, :, None].broadcast_to([P, NT, H]), Alu.is_gt)
            nc.vector.tensor_tensor(row_nb[:], row_nb[:], row_gt[:], Alu.subtract)
            if EXACT:
                # row mask scaled by -sel*lam so out = (q + a) * x
                nc.vector.tensor_tensor(row_nb[:], row_nb[:], nb_t[:, :, None].broadcast_to([P, NT, H]), Alu.mult)
            else:
                # row mask gated by sel so out = (q == 0) * y
                nc.vector.tensor_tensor(row_nb[:], row_nb[:], selp[:, :, None].broadcast_to([P, NT, H]), Alu.mult)

            # ---- outer products, KQ tiles per instruction, POOL/DVE split ----
            assert NT % KQ == 0
            NB = NT // KQ
            qb = []
            for b_ in range(NB):
                q = qpool.tile([P, KQ, H, W], mdt, name=f"q{b_}", tag="q")
                eng = nc.gpsimd if b_ < POOL_QBATCHES else nc.vector
                t0 = b_ * KQ
                eng.tensor_tensor(
                    q[:],
                    row_nb[:, t0:t0 + KQ, :, None].broadcast_to([P, KQ, H, W]),
                    col_in[:, t0:t0 + KQ, None, :].broadcast_to([P, KQ, H, W]),
                    Alu.mult,
                )
                qb.append(q)

            def q_tile(t):
                b_, j = divmod(t, KQ)
                return qb[b_][:, j]

            # ---- apply + output DMA, chunked ----
            for c in range(NCHUNK):
                o_c = opool.tile([P, CH_T * HW], odt, name=f"o{c}", tag="oc")
                for j in range(CH_T):
                    t = c * CH_T + j
                    o_t = o_c[:, j * HW:(j + 1) * HW].rearrange("p (h w) -> p h w", h=H, w=W)
                    if EXACT:
                        # out = (q + a) * x,  q = -sel*lam*box
                        nc.vector.scalar_tensor_tensor(
                            o_t, q_tile(t), a_t[:, t, None],
                            x_tile(t).rearrange("p (h w) -> p h w", h=H, w=W),
                            Alu.add, Alu.mult,
                        )
                    else:
                        # y = a*x on ScalarE (bf16 out), out = (q==0)*y on DVE
                        y = ypool.tile([P, H, W], bf16, name=f"y{t}", tag="y")
                        nc.scalar.activation(
                            y.rearrange("p h w -> p (h w)"), x_tile(t),
                            Act.Copy, bias=0.0, scale=a_t[:, t, None],
                        )
                        nc.vector.scalar_tensor_tensor(
                            o_t, q_tile(t), 0.0, y[:],
                            Alu.is_equal, Alu.mult,
                        )
                nc.sync.dma_start(out_d[:, c * CH_T * HW:(c + 1) * CH_T * HW], o_c[:])

    nc.compile()
    return nc


def _get_nc(half: int):
    if half not in _cached:
        _cached[half] = _build(half)
    return _cached[half]


def _shard_inputs(x, T):
    xf = np.ascontiguousarray(x, dtype=np.float32).reshape(-1, HW)   # [32768, 1024]
    sel = (np.asarray(T).reshape(-1) != 0).astype(np.float32)        # [32768]
    io32 = np.tile(np.arange(32, dtype=np.float32), (P, 1))
    in_maps = []
    for i in range(N_CORES):
        lo = i * SLICES_PER_CORE
        hi = lo + SLICES_PER_CORE
        in_maps.append({
            "x": np.ascontiguousarray(xf[lo:hi].reshape(P, NT * HW)),
            "sel": np.ascontiguousarray(sel[lo:hi].reshape(P, NT)),
            "io32": io32,
        })
    return in_maps


def run(inputs, trace=False, **kw):
    x = inputs["x"]
    T = inputs["T"]
    drop_block = int(np.asarray(inputs["drop_block"]))
    half = drop_block // 2
    b, c, h, w = x.shape
    assert (h, w) == (H, W) and b * c == N_CORES * SLICES_PER_CORE, \
        f"kernel hardcoded for (128,256,32,32); got {x.shape}"

    nc = _get_nc(half)
    in_maps = _shard_inputs(x, T)
    res = run_bass_kernel_spmd(nc, in_maps, core_ids=list(range(N_CORES)),
                               trace=trace, **kw)
    parts = [np.asarray(res.results[i]["out"]).astype(np.float32)
              .reshape(SLICES_PER_CORE, HW)
             for i in range(N_CORES)]
    out = np.concatenate(parts, axis=0).reshape(b, c, h, w)
    return out, res


def kernel(**inputs) -> np.ndarray:
    out, _ = run(inputs, trace=False)
    return out
